# revision 17
# baseline (speedup 1.0000x reference)
"""Trainium2 Bass kernel for nn_CustomLSTM: B=32, S=512, D=512, H=1024.

Strategy v3 (8 NeuronCores = 2 dies x 4 cores):
  - DP=2 over batch across the two dies (cores 0-3 take batch 0:16,
    cores 4-7 take batch 16:32). The LSTM recurrence is batch-independent,
    so the two dies never communicate: this removes the slow cross-die
    (D2D) hops from the per-step h exchange that dominated v2's period.
  - TP=4 over hidden units within each die: each core owns 256 units
    (2 tiles of 128), computes all four gates for those units, and
    broadcasts its h shard (bf16, [128 part x 32 cols]) to its die-local
    group (itself + 3 peers) each step via remote_dma_broadcast with 16
    slots -> 4 DMA engines per destination (halves the descriptor drain).
  - Per step: 64 recurrent bf16 matmuls (4 gates x 2 out-tiles x 8
    contraction chunks, N=16) accumulate into one PSUM bank laid out
    [f|g|i|o] x (u_hi, b); the 32 x-projection matmuls for step t+1 are
    issued first so they fill the arrival-wait window. ACT applies
    sigmoid/tanh per gate; DVE forms c and h.
  - The DMA trigger fires as soon as tanh(c) completes: the ~0.9us SWDGE
    doorbell latency covers the remaining h=o*tanh(c) DVE op (~0.25us
    margin measured on the v2 trace), so the send overlaps the tail.
  - Raw bass, hand-scheduled engine programs with explicit semaphores.

Self-contained: hardcodes all shapes; host side shards/reassembles.
"""
import numpy as np
import ml_dtypes

import concourse.bacc as bacc
import concourse.mybir as mybir
from concourse import bass_utils

F32 = mybir.dt.float32
BF16 = mybir.dt.bfloat16

B, S, D, H = 32, 512, 512, 1024
NCORES = 8
GROUP = 4                  # tensor-parallel group size (per die)
B2 = B // 2                # batch per die = 16
UC = H // GROUP            # units per core = 256 (2 tiles of 128)

import os
if os.environ.get("KERNEL_SIM_STEPS"):
    S = int(os.environ["KERNEL_SIM_STEPS"])
ROWS = S * B2              # x rows per core = 8192, (s, b) order
NB = (ROWS + 511) // 512   # x row blocks of 512 rows (32 steps each)
NBLK = (S + 15) // 16      # output blocks of 16 steps
SAFE_TRIGGER = bool(os.environ.get("KERNEL_SAFE_TRIGGER"))

_cache = {}

# gate order everywhere: f(0), g(1), i(2), o(3); PSUM cols g*32 + u_hi*16 + b


def _build(zero_bias=True):
    # NB: the CoreSim race detector cannot model the cross-core transitive
    # ordering through remote-DMA semaphores that this kernel (like v2)
    # relies on, so it stays off.
    nc = bacc.Bacc(None, target_bir_lowering=False, num_devices=NCORES,
                   detect_race_conditions=False)

    # ---------------- DRAM I/O ----------------
    xT_d = nc.dram_tensor("xT", [128, 4, ROWS], BF16, kind="ExternalInput")
    w_in_d = nc.dram_tensor("w_in", [128, 4, 8, 128], BF16, kind="ExternalInput")
    w_rec_d = nc.dram_tensor("w_rec", [128, 8, 8, 128], BF16, kind="ExternalInput")
    bias_d = nc.dram_tensor("bias", [128, 8], F32, kind="ExternalInput")
    h_out_d = nc.dram_tensor("h_out", [NBLK, 128, 16, 32], F32, kind="ExternalOutput")
    c_out_d = nc.dram_tensor("c_out", [NBLK, 128, 16, 32], F32, kind="ExternalOutput")

    # ---------------- SBUF ----------------
    xt_buf = nc.alloc_sbuf_tensor("xt_buf", [128, 3, 4, 512], BF16)
    w_in_sb = nc.alloc_sbuf_tensor("w_in_sb", [128, 4, 8, 128], BF16)
    w_rec_sb = nc.alloc_sbuf_tensor("w_rec_sb", [128, 8, 8, 128], BF16)
    bias_sb = nc.alloc_sbuf_tensor("bias_sb", [128, 8], F32)
    # per-gate activations, parity-buffered: f stays f32 (multiplies f32 c)
    f_act = nc.alloc_sbuf_tensor("f_act", [128, 2, 32], F32)
    gact = nc.alloc_sbuf_tensor("gact", [128, 2, 3, 32], BF16)  # g(0), i(1), o(2)
    tmpu = nc.alloc_sbuf_tensor("tmpu", [128, 32], F32)
    tmpv = nc.alloc_sbuf_tensor("tmpv", [128, 32], F32)
    tc_sb = nc.alloc_sbuf_tensor("tc_sb", [128, 2, 32], BF16)
    c_ring = nc.alloc_sbuf_tensor("c_ring", [128, 16, 32], F32)
    h_ring = nc.alloc_sbuf_tensor("h_ring", [128, 16, 32], F32)
    # received h shards: [parity, sender rank(4), u_hi(2) x b(16) = 32]
    hT_buf = nc.alloc_sbuf_tensor("hT_buf", [128, 2, 4, 32], BF16)
    h_send = nc.alloc_sbuf_tensor("h_send", [128, 2, 32], BF16)

    # bank g*2+u holds (gate g, out-tile u), cols 0:16 = batch. One
    # accumulation group per bank (start=True claims the whole 2KB bank);
    # exclusion across steps comes from xpre(t+1) waiting on ACT(t).
    pp = nc.alloc_psum_tensor("pp", [128, 8, 512], F32)

    # ---------------- semaphores ----------------
    arr = [nc.alloc_semaphore(f"arr{j}") for j in range(GROUP)]
    loc_sem = nc.alloc_semaphore("loc")
    prep_sem = nc.alloc_semaphore("prep")
    dma_w = nc.alloc_semaphore("dma_w")
    xt_sem = nc.alloc_semaphore("xt_sem")
    xtf = nc.alloc_semaphore("xtf")          # x block consumed by PE
    pg = nc.alloc_semaphore("pg")            # psum gate done: 4 per step
    acts = nc.alloc_semaphore("acts")        # ACT gate ops done: 4 per step
    tcs = nc.alloc_semaphore("tcs")          # tanh(c) done: 1 per step
    cr = nc.alloc_semaphore("cr")            # c written to ring: 1 per step
    hr = nc.alloc_semaphore("hr")            # h staged to ring: 1 per step
    h_ready = nc.alloc_semaphore("h_ready")  # h_send written: 1 per step
    outc_sem = nc.alloc_semaphore("outc_sem")
    outh_sem = nc.alloc_semaphore("outh_sem")

    w_in = w_in_sb.ap()
    w_rec = w_rec_sb.ap()
    ppa = pp.ap()
    hT = hT_buf.ap()
    ga = gact.ap()

    def xpre_mms(pe, t, first_step=False):
        """x-projection matmuls for step t into bank t%4."""
        blk, r = t // 32, t % 32
        if r == 0:
            pe.wait_ge(xt_sem, 64 * (blk + 1))
        for g in range(4):
            last = None
            for u in range(2):
                for kx in range(4):
                    last = nc.tensor.matmul(
                        ppa[:, g * 2 + u, 0:16],
                        w_in[:, kx, g * 2 + u, :],
                        xt_buf.ap()[:, blk % 3, kx, r * 16:(r + 1) * 16],
                        start=(kx == 0),
                        stop=(first_step and kx == 3),
                    )
            if first_step:
                last.then_inc(pg, 1)
        if r == 31:
            last.then_inc(xtf, 1)

    with nc.Block() as block:

        # ================= SP: DMA feeder + batched output writer =========
        @block.sync
        def _(sp):
            sp.dma_start(w_in_sb.ap(), w_in_d.ap()).then_inc(dma_w, 16)
            sp.dma_start(w_rec_sb.ap(), w_rec_d.ap()).then_inc(dma_w, 16)
            sp.dma_start(bias_sb.ap(), bias_d.ap()).then_inc(dma_w, 16)
            for n in range(min(3, NB)):
                rows = min(512, ROWS - n * 512)
                for q in range(4):
                    sp.dma_start(
                        xt_buf.ap()[:, n, q, 0:rows],
                        xT_d.ap()[:, q, n * 512:n * 512 + rows],
                    ).then_inc(xt_sem, 16)
            for b in range(NBLK):
                sp.wait_ge(cr, min(16 * b + 16, S))
                sp.dma_start(c_out_d.ap()[b], c_ring.ap()).then_inc(outc_sem, 16)
                sp.wait_ge(hr, min(16 * b + 16, S))
                sp.dma_start(h_out_d.ap()[b], h_ring.ap()).then_inc(outh_sem, 16)
                # x blocks are 32 steps (2 output blocks): prefetch block
                # b//2+3 into slot (b//2+3)%3 once block b//2 is consumed,
                # quarters spread across tail windows to stay clear of the
                # broadcast flights.
                if b % 2 == 0:
                    n = b // 2 + 3
                    if n < NB:
                        sp.wait_ge(xtf, n - 2)
                        for q in range(4):
                            sp.wait_ge(cr, 16 * b + 22 + 2 * q)
                            sp.dma_start(
                                xt_buf.ap()[:, n % 3, q, :],
                                xT_d.ap()[:, q, n * 512:(n + 1) * 512],
                            ).then_inc(xt_sem, 16)

        # ================= PE =================
        @block.tensor
        def _(pe):
            pe.wait_ge(dma_w, 48)
            # settle delay: avoid racing device/engine startup (see v2)
            if not os.environ.get("KERNEL_SIM_STEPS"):
                for _ in range(2):
                    pe.nop(cycle_cnt=60000)
            # prologue: x-projection for step 0 (its own start/stop group)
            xpre_mms(pe, 0, first_step=True)
            for t in range(1, S):
                # x-projection for step t: banks were last read by ACT(t-1)
                pe.wait_ge(acts, 4 * (t - 1) + 4)
                xpre_mms(pe, t)
                # recurrent matmuls, gate-major f,g,i,o
                for g in range(4):
                    ins = None
                    for srank in range(GROUP):
                        if g == 0:
                            pe.wait_ge(arr[srank], 2 * t)
                        for u_in in range(2):
                            for u in range(2):
                                ins = nc.tensor.matmul(
                                    ppa[:, g * 2 + u, 0:16],
                                    w_rec[:, 2 * srank + u_in, g * 2 + u, :],
                                    hT[:, (t - 1) % 2, srank, u_in * 16:(u_in + 1) * 16],
                                    start=False,
                                    stop=(srank == GROUP - 1 and u_in == 1),
                                )
                    ins.then_inc(pg, 1)

        # ================= ACT =================
        @block.scalar
        def _(act):
            funcs = [
                mybir.ActivationFunctionType.Sigmoid,  # f
                mybir.ActivationFunctionType.Tanh,     # g
                mybir.ActivationFunctionType.Sigmoid,  # i
                mybir.ActivationFunctionType.Sigmoid,  # o
            ]
            for t in range(S):
                p = t % 2
                for g in range(4):
                    act.wait_ge(pg, 4 * t + g + 1)
                    out = f_act.ap()[:, p, :] if g == 0 else ga[:, p, g - 1, :]
                    if zero_bias:
                        # one op spanning the gate's two banks (strided read)
                        nc.scalar.activation(
                            out, ppa[:, g * 2:g * 2 + 2, 0:16], funcs[g]
                        ).then_inc(acts, 1)
                    else:
                        # bias differs between the two u_hi halves: 2 ops
                        for u in range(2):
                            ins = nc.scalar.activation(
                                out[:, u * 16:(u + 1) * 16],
                                ppa[:, g * 2 + u, 0:16],
                                funcs[g],
                                bias=bias_sb.ap()[:, g * 2 + u:g * 2 + u + 1],
                            )
                        ins.then_inc(acts, 1)
                act.wait_ge(cr, t + 1)
                nc.scalar.activation(
                    tc_sb.ap()[:, p, :],
                    c_ring.ap()[:, t % 16, :],
                    mybir.ActivationFunctionType.Tanh,
                ).then_inc(tcs, 1)

        # ================= DVE =================
        @block.vector
        def _(dve):
            dve.memset(c_ring.ap()[:, 15, :], 0.0)
            if S < 16:  # sim only: output DMA reads the full ring
                dve.memset(c_ring.ap(), 0.0)
                dve.memset(h_ring.ap(), 0.0)
            dve.drain()
            for t in range(S):
                p = t % 2
                if t % 16 == 0 and t >= 16:
                    dve.wait_ge(outc_sem, 16 * (t // 16))
                # v = f * c_prev   (ACT order: f is 1st)
                dve.wait_ge(acts, 4 * t + 1)
                nc.vector.tensor_mul(
                    tmpv.ap(), f_act.ap()[:, p, :], c_ring.ap()[:, (t - 1) % 16, :]
                )
                # u = i * g~   (g 2nd, i 3rd)
                dve.wait_ge(acts, 4 * t + 3)
                nc.vector.tensor_mul(tmpu.ap(), ga[:, p, 1, :], ga[:, p, 0, :])
                dve.drain()
                # c_t = u + v
                nc.vector.tensor_add(
                    c_ring.ap()[:, t % 16, :], tmpu.ap(), tmpv.ap()
                ).then_inc(cr, 1)
                # h_send = o * tanh(c) (bf16) for the broadcast
                if t < S - 1:
                    if t >= 2:
                        dve.wait_ge(loc_sem, 16 * (t - 1))
                    dve.wait_ge(acts, 4 * t + 4)
                    dve.wait_ge(tcs, t + 1)
                    nc.vector.tensor_mul(
                        h_send.ap()[:, p, :],
                        ga[:, p, 2, :],
                        tc_sb.ap()[:, p, :],
                    ).then_inc(h_ready, 1)
                # stage h_t to the output ring (f32), off the critical path
                if t % 16 == 0 and t >= 16:
                    dve.wait_ge(outh_sem, 16 * (t // 16))
                dve.wait_ge(acts, 4 * t + 4)
                dve.wait_ge(tcs, t + 1)
                nc.vector.tensor_mul(
                    h_ring.ap()[:, t % 16, :],
                    ga[:, p, 2, :],
                    tc_sb.ap()[:, p, :],
                ).then_inc(hr, 1)

        # ================= Pool: die-local broadcast (incl. self) =========
        @block.gpsimd
        def _(g):
            g.bir_kernel_barrier_wait([list(range(NCORES))])
            pid_reg = g.to_reg(g.partition_id())
            # 8 slots: 4 die-local dests (self + 3 peers), 2 lanes each;
            # each dest's remote sem gets +2 per send
            slots = [(0, d) for d in range(4)] + [None] * 4
            for kcore in range(NCORES):
                myrank = kcore % GROUP
                with g.If_eq(pid_reg, kcore):
                    for t in range(S - 1):
                        if t >= 1:
                            # descriptor-carveout reclaim: broadcast t-1's
                            # descs must be consumed before regenerating
                            g.wait_ge(loc_sem, 16 * t)
                        g.remote_dma_broadcast(
                            out_ap=hT[:, t % 2, myrank, :],
                            in_ap=h_send.ap()[:, t % 2, :],
                            remote_sem=arr[myrank],
                            local_sem=loc_sem,
                            rdests=slots,
                        ).then_inc(prep_sem, 1)
                        g.wait_ge(prep_sem, t + 1)
                        if SAFE_TRIGGER:
                            g.wait_ge(h_ready, t + 1)
                        else:
                            # tanh(c_t) done; the SWDGE doorbell latency
                            # (~0.9us) covers the h=o*tanh(c) DVE op
                            g.wait_ge(tcs, t + 1)
                        g.trigger_dma(1)

    nc.finalize()
    return nc


def _prep_inputs(x, W_ii, W_if, W_ig, W_io, W_hi, W_hf, W_hg, W_ho,
                 b_i, b_f, b_g, b_o):
    bf = ml_dtypes.bfloat16
    Wx = [W_if, W_ig, W_ii, W_io]   # gate order f, g, i, o
    Wh = [W_hf, W_hg, W_hi, W_ho]
    bs = [b_f, b_g, b_i, b_o]

    # per-die x: [D, S, B2] -> [4, 128, S*B2] -> [128, 4, S*B2]; rows (s, b)
    xTs = []
    for d in range(2):
        xd = np.asarray(x[B2 * d:B2 * (d + 1)])          # [16, S, D]
        xT = np.ascontiguousarray(
            xd.transpose(2, 1, 0).reshape(4, 128, ROWS).transpose(1, 0, 2)
        ).astype(bf)
        xTs.append(xT)

    in_maps = []
    for c in range(NCORES):
        rank = c % GROUP
        U0 = UC * rank
        w_rec_c = np.empty((128, 8, 8, 128), np.float32)
        w_in_c = np.empty((128, 4, 8, 128), np.float32)
        bias_c = np.empty((128, 8), np.float32)
        for g in range(4):
            for u in range(2):
                cols = slice(U0 + 128 * u, U0 + 128 * (u + 1))
                for k in range(8):
                    w_rec_c[:, k, g * 2 + u, :] = Wh[g][128 * k:128 * (k + 1), cols]
                for kx in range(4):
                    w_in_c[:, kx, g * 2 + u, :] = Wx[g][128 * kx:128 * (kx + 1), cols]
                bias_c[:, g * 2 + u] = bs[g][cols]
        in_maps.append({
            "xT": xTs[c // GROUP],
            "w_in": np.ascontiguousarray(w_in_c.astype(bf)),
            "w_rec": np.ascontiguousarray(w_rec_c.astype(bf)),
            "bias": np.ascontiguousarray(bias_c),
        })
    return in_maps


def run(inputs, trace=False, trace_cores=None):
    zero_bias = all(
        not np.any(np.asarray(inputs[k])) for k in ("b_i", "b_f", "b_g", "b_o")
    )
    key = ("nc", zero_bias)
    if key not in _cache:
        _cache[key] = _build(zero_bias=zero_bias)
    nc = _cache[key]
    in_maps = _prep_inputs(**inputs)
    kw = {}
    if trace_cores is not None:
        kw["trace_cores"] = trace_cores
    res = bass_utils.run_bass_kernel_spmd(
        nc, in_maps, core_ids=list(range(NCORES)), trace=trace, **kw
    )
    outputs = np.empty((B, S, H), np.float32)
    cells = np.empty((B, S, H), np.float32)
    for c in range(NCORES):
        d, rank = c // GROUP, c % GROUP
        U = slice(UC * rank, UC * (rank + 1))
        Bs = slice(B2 * d, B2 * (d + 1))
        # [NBLK, 128(p), 16(r), 2(u_hi), 16(b)] -> [b, s, u=u_hi*128+p]
        h = res.results[c]["h_out"].reshape(NBLK, 128, 16, 2, 16)
        cc = res.results[c]["c_out"].reshape(NBLK, 128, 16, 2, 16)
        outputs[Bs, :, U] = h.transpose(4, 0, 2, 3, 1).reshape(B2, S, UC)
        cells[Bs, :, U] = cc.transpose(4, 0, 2, 3, 1).reshape(B2, S, UC)
    return (outputs, cells), res


def kernel(**inputs):
    (outputs, cells), _ = run(inputs, trace=False)
    return outputs, cells


# revision 19
# speedup vs baseline: 1.0753x; 1.0753x over previous
"""Trainium2 Bass kernel for nn_CustomLSTM: B=32, S=512, D=512, H=1024.

Strategy (8 NeuronCores, one chip), v2:
  - Tensor-parallel over hidden units: core c owns H-units [128c, 128c+128)
    and computes all four gates for those units (gate order g, i, f, o).
  - No separate input-projection phase: each step's x-projection (4 gates x
    4 k-chunks, N=32) is computed by the PE into the step's PSUM bank during
    the previous step's broadcast dead time; x is streamed from DRAM in
    512-row blocks (16 steps per block). Biases fold into the ACT
    activations (per-partition bias operand).
  - Per step: 32 recurrent bf16 matmuls accumulate W_h^T @ h_{t-1} into one
    PSUM bank ([128 units, 4 gates x 32 batch]), gate-major so ACT overlaps
    the PE; ACT applies tanh/sigmoid per gate; DVE forms c and h; h (bf16)
    is exchanged between the 8 cores with remote_dma_broadcast.
  - Outputs (h, c) accumulate in 16-step SBUF rings and are written to DRAM
    once per 16 steps so the broadcast flight window stays clear of DMA
    descriptor traffic.
  - Raw bass (no Tile): hand-scheduled engine programs with explicit
    semaphores.

Self-contained: hardcodes all shapes; host side shards/reassembles.
"""
import numpy as np
import ml_dtypes

import concourse.bacc as bacc
import concourse.mybir as mybir
from concourse import bass_utils

F32 = mybir.dt.float32
BF16 = mybir.dt.bfloat16

B, S, D, H = 32, 512, 512, 1024
NCORES = 8
UC = H // NCORES          # units per core = 128

import os
if os.environ.get("KERNEL_SIM_STEPS"):
    S = int(os.environ["KERNEL_SIM_STEPS"])
NB = (S * B + 511) // 512   # x row blocks of 512 rows (16 steps each)
NBLK = (S + 15) // 16       # output blocks of 16 steps
ROWS = S * B

_cache = {}

# gate order everywhere: g(0), i(1), f(2), o(3)


def _build(detect_races=True):
    nc = bacc.Bacc(None, target_bir_lowering=False, num_devices=NCORES,
                   detect_race_conditions=detect_races)

    # ---------------- DRAM I/O ----------------
    xT_d = nc.dram_tensor("xT", [128, 4, ROWS], BF16, kind="ExternalInput")
    w_in_d = nc.dram_tensor("w_in", [128, 4, 512], BF16, kind="ExternalInput")
    w_rec_d = nc.dram_tensor("w_rec", [128, 8, 512], BF16, kind="ExternalInput")
    bias_d = nc.dram_tensor("bias", [128, 4], F32, kind="ExternalInput")
    h_out_d = nc.dram_tensor("h_out", [NBLK, 128, 16, 32], F32, kind="ExternalOutput")
    c_out_d = nc.dram_tensor("c_out", [NBLK, 128, 16, 32], F32, kind="ExternalOutput")

    # ---------------- SBUF ----------------
    xt_buf = nc.alloc_sbuf_tensor("xt_buf", [128, 3, 4, 512], BF16)
    w_in_sb = nc.alloc_sbuf_tensor("w_in_sb", [128, 4, 512], BF16)
    w_rec_sb = nc.alloc_sbuf_tensor("w_rec_sb", [128, 8, 512], BF16)
    bias_sb = nc.alloc_sbuf_tensor("bias_sb", [128, 4], F32)
    gact = nc.alloc_sbuf_tensor("gact", [128, 2, 4, 32], BF16)
    f_act = nc.alloc_sbuf_tensor("f_act", [128, 2, 32], F32)
    tmpu = nc.alloc_sbuf_tensor("tmpu", [128, 32], F32)
    tmpv = nc.alloc_sbuf_tensor("tmpv", [128, 32], F32)
    tc_sb = nc.alloc_sbuf_tensor("tc_sb", [128, 2, 32], BF16)
    c_ring = nc.alloc_sbuf_tensor("c_ring", [128, 16, 32], F32)
    h_ring = nc.alloc_sbuf_tensor("h_ring", [128, 16, 32], F32)
    hT_buf = nc.alloc_sbuf_tensor("hT_buf", [128, 2, 8, 32], BF16)
    h_send = nc.alloc_sbuf_tensor("h_send", [128, 2, 32], BF16)

    # bank p*4+g holds gate g of step parity p (cols 0:32)
    pp = nc.alloc_psum_tensor("pp", [128, 8, 512], F32)

    # ---------------- semaphores ----------------
    arr = [nc.alloc_semaphore(f"arr{j}") for j in range(NCORES)]
    loc_sem = nc.alloc_semaphore("loc")
    prep_sem = nc.alloc_semaphore("prep")
    dma_w = nc.alloc_semaphore("dma_w")
    xt_sem = nc.alloc_semaphore("xt_sem")
    xtf = nc.alloc_semaphore("xtf")          # x block consumed by PE
    pg = nc.alloc_semaphore("pg")            # psum gate done: 4 per step
    acts = nc.alloc_semaphore("acts")        # ACT gate ops done: 4 per step
    tcs = nc.alloc_semaphore("tcs")          # tanh(c) done: 1 per step
    cr = nc.alloc_semaphore("cr")            # c written to ring: 1 per step
    hr = nc.alloc_semaphore("hr")            # h staged to ring: 1 per step
    h_ready = nc.alloc_semaphore("h_ready")  # h_send written: 1 per step
    outc_sem = nc.alloc_semaphore("outc_sem")
    outh_sem = nc.alloc_semaphore("outh_sem")

    w_in = w_in_sb.ap()
    w_rec = w_rec_sb.ap()
    ppa = pp.ap()
    hT = hT_buf.ap()
    ga = gact.ap()

    def xpre_mms(pe, t):
        """x-projection matmuls for step t into bank t%2 (start of group)."""
        blk, r = t // 16, t % 16
        if r == 0:
            pe.wait_ge(xt_sem, 64 * (blk + 1))
        p = t % 2
        last = None
        for g in (0, 2, 1, 3):
            for k in range(4):
                last = nc.tensor.matmul(
                    ppa[:, p * 4 + g, 0:32],
                    w_in[:, k, g * 128:(g + 1) * 128],
                    xt_buf.ap()[:, blk % 3, k, r * 32:(r + 1) * 32],
                    start=(k == 0),
                    stop=(t == 0 and k == 3),
                )
            if t == 0:
                last.then_inc(pg, 1)
        if r == 15:
            last.then_inc(xtf, 1)

    with nc.Block() as block:

        # ================= SP: DMA feeder + batched output writer =========
        @block.sync
        def _(sp):
            sp.dma_start(w_in_sb.ap(), w_in_d.ap()).then_inc(dma_w, 16)
            sp.dma_start(w_rec_sb.ap(), w_rec_d.ap()).then_inc(dma_w, 16)
            sp.dma_start(bias_sb.ap(), bias_d.ap()).then_inc(dma_w, 16)
            for n in range(min(3, NB)):
                for q in range(4):
                    sp.dma_start(
                        xt_buf.ap()[:, n, q, :],
                        xT_d.ap()[:, q, n * 512:(n + 1) * 512],
                    ).then_inc(xt_sem, 16)
            for b in range(NBLK):
                sp.wait_ge(cr, 16 * b + 16)
                sp.dma_start(c_out_d.ap()[b], c_ring.ap()).then_inc(outc_sem, 16)
                sp.wait_ge(hr, 16 * b + 16)
                sp.dma_start(h_out_d.ap()[b], h_ring.ap()).then_inc(outh_sem, 16)
                # xt block b+3 reuses block b's slot (consumed at t=16b+14).
                # The 512KB load occupies the DMA engines ~1.4us, so split it
                # into 4 quarter-loads spread over 4 tail windows (t≡5,7,9,11)
                # to keep it out of the broadcast flights.
                n = b + 3
                if n < NB:
                    sp.wait_ge(xtf, b + 1)
                    for q in range(4):
                        sp.wait_ge(cr, 16 * b + 22 + 2 * q)
                        sp.dma_start(
                            xt_buf.ap()[:, n % 3, q, :],
                            xT_d.ap()[:, q, n * 512:(n + 1) * 512],
                        ).then_inc(xt_sem, 16)

        # ================= PE =================
        @block.tensor
        def _(pe):
            pe.wait_ge(dma_w, 48)
            # settle delay: the first broadcast otherwise fires ~5us after
            # kernel start and races device/engine startup, corrupting the
            # first steps (the old phase-1 gave the baseline ~250us of grace)
            if not os.environ.get("KERNEL_SIM_STEPS"):
                for _ in range(2):
                    pe.nop(cycle_cnt=60000)
            # prologue: x-projections for steps 0 and 1
            xpre_mms(pe, 0)
            if S > 1:
                xpre_mms(pe, 1)
            for t in range(1, S):
                p = t % 2
                # recurrent matmuls, issue order g, f, i, o: f before i
                # so the DVE's v=f*c_prev starts one ACT op earlier
                for g in (0, 2, 1, 3):
                    for k in range(NCORES):
                        if g == 0:
                            pe.wait_ge(arr[k], 2 * t)
                        ins = nc.tensor.matmul(
                            ppa[:, p * 4 + g, 0:32],
                            w_rec[:, k, g * 128:(g + 1) * 128],
                            hT[:, (t - 1) % 2, k, :],
                            start=False,
                            stop=(k == NCORES - 1),
                        )
                    ins.then_inc(pg, 1)
                # x-projection for step t+1 (runs in step t's dead time);
                # bank (t+1)%2 must have been drained by ACT at step t-1
                if t + 1 < S:
                    pe.wait_ge(acts, 4 * (t - 1) + 4)
                    xpre_mms(pe, t + 1)

        # ================= ACT =================
        @block.scalar
        def _(act):
            funcs = {
                0: mybir.ActivationFunctionType.Tanh,     # g
                1: mybir.ActivationFunctionType.Sigmoid,  # i
                2: mybir.ActivationFunctionType.Sigmoid,  # f
                3: mybir.ActivationFunctionType.Sigmoid,  # o
            }
            for t in range(S):
                p = t % 2
                for pos, g in enumerate((0, 2, 1, 3)):
                    act.wait_ge(pg, 4 * t + pos + 1)
                    # f (g==2) stays f32: it multiplies the f32 c_prev on
                    # DVE and mixed-dtype DVE inputs are risky on HW
                    out = f_act.ap()[:, p, :] if g == 2 else ga[:, p, g, :]
                    nc.scalar.activation(
                        out,
                        ppa[:, p * 4 + g, 0:32],
                        funcs[g],
                        bias=bias_sb.ap()[:, g:g + 1],
                    ).then_inc(acts, 1)
                act.wait_ge(cr, t + 1)
                nc.scalar.activation(
                    tc_sb.ap()[:, t % 2, :],
                    c_ring.ap()[:, t % 16, :],
                    mybir.ActivationFunctionType.Tanh,
                ).then_inc(tcs, 1)

        # ================= DVE =================
        @block.vector
        def _(dve):
            dve.memset(c_ring.ap()[:, 15, :], 0.0)
            dve.drain()
            for t in range(S):
                p = t % 2
                if t % 16 == 0 and t >= 16:
                    dve.wait_ge(outc_sem, 16 * (t // 16))
                # stage h_{t-1} = o_{t-1} * tanh(c_{t-1}) to the ring
                if t >= 1:
                    if (t - 1) % 16 == 0 and t - 1 >= 16:
                        dve.wait_ge(outh_sem, 16 * ((t - 1) // 16))
                    dve.wait_ge(tcs, t)
                    nc.vector.tensor_mul(
                        h_ring.ap()[:, (t - 1) % 16, :],
                        ga[:, (t - 1) % 2, 3, :],
                        tc_sb.ap()[:, (t - 1) % 2, :],
                    ).then_inc(hr, 1)
                # v = f * c_prev   (acts: g~ is 1st, sigma-f is 2nd)
                dve.wait_ge(acts, 4 * t + 2)
                nc.vector.tensor_mul(
                    tmpv.ap(), f_act.ap()[:, p, :], c_ring.ap()[:, (t - 1) % 16, :]
                )
                # u = i * g~   (sigma-i is 3rd)
                dve.wait_ge(acts, 4 * t + 3)
                nc.vector.tensor_mul(tmpu.ap(), ga[:, p, 1, :], ga[:, p, 0, :])
                dve.drain()
                # c_t = u + v
                nc.vector.tensor_add(
                    c_ring.ap()[:, t % 16, :], tmpu.ap(), tmpv.ap()
                ).then_inc(cr, 1)
                # h_send = o * tanh(c) (bf16) for the broadcast
                if t < S - 1:
                    if t >= 2:
                        dve.wait_ge(loc_sem, 16 * (t - 1))
                    dve.wait_ge(acts, 4 * t + 4)
                    dve.wait_ge(tcs, t + 1)
                    nc.vector.tensor_mul(
                        h_send.ap()[:, p, :], ga[:, p, 3, :], tc_sb.ap()[:, p, :]
                    ).then_inc(h_ready, 1)
            # epilogue: stage h_{S-1}
            dve.wait_ge(tcs, S)
            nc.vector.tensor_mul(
                h_ring.ap()[:, (S - 1) % 16, :],
                ga[:, (S - 1) % 2, 3, :],
                tc_sb.ap()[:, (S - 1) % 2, :],
            ).then_inc(hr, 1)

        # ================= Pool: remote all-gather =================
        @block.gpsimd
        def _(g):
            g.bir_kernel_barrier_wait([list(range(NCORES))])
            pid_reg = g.to_reg(g.partition_id())
            for kcore in range(NCORES):
                with g.If_eq(pid_reg, kcore):
                    for t in range(S - 1):
                        if t >= 1:
                            # descriptor-carveout reclaim: broadcast t-1's
                            # descs must be consumed before regenerating
                            g.wait_ge(loc_sem, 16 * t)
                        g.remote_dma_broadcast(
                            out_ap=hT[:, t % 2, kcore, :],
                            in_ap=h_send.ap()[:, t % 2, :],
                            remote_sem=arr[kcore],
                            local_sem=loc_sem,
                            rdests=[(0, j) for j in range(NCORES)],
                        ).then_inc(prep_sem, 1)
                        g.wait_ge(prep_sem, t + 1)
                        # fire on tanh(c_t) completion: the ~0.68us SWDGE
                        # doorbell latency covers the h=o*tanh(c) DVE op
                        # (~0.34us), so the send overlaps the chain tail.
                        # (verified race-free margin on HW traces)
                        g.wait_ge(tcs, t + 1)
                        g.trigger_dma(1)

    nc.finalize()
    return nc


def _prep_inputs(x, W_ii, W_if, W_ig, W_io, W_hi, W_hf, W_hg, W_ho,
                 b_i, b_f, b_g, b_o):
    bf = ml_dtypes.bfloat16
    # xT: [D, S, B] -> [4, 128, S*B] -> [128, 4, S*B]; rows ordered (s, b)
    xT = np.ascontiguousarray(
        x.transpose(2, 1, 0).reshape(4, 128, ROWS).transpose(1, 0, 2)
    ).astype(bf)

    in_maps = []
    for c in range(NCORES):
        U = slice(UC * c, UC * (c + 1))
        # gate order [g, i, f, o]
        w_in_c = np.concatenate(
            [W_ig[:, U], W_ii[:, U], W_if[:, U], W_io[:, U]], axis=1
        )  # [512, 512]
        w_in_c = w_in_c.reshape(4, 128, 512).transpose(1, 0, 2).astype(bf)
        w_rec_c = np.concatenate(
            [W_hg[:, U], W_hi[:, U], W_hf[:, U], W_ho[:, U]], axis=1
        )  # [1024, 512]
        w_rec_c = w_rec_c.reshape(8, 128, 512).transpose(1, 0, 2).astype(bf)
        bias_c = np.stack(
            [b_g[U], b_i[U], b_f[U], b_o[U]], axis=1
        ).astype(np.float32)  # [128, 4]
        in_maps.append({
            "xT": xT,
            "w_in": np.ascontiguousarray(w_in_c),
            "w_rec": np.ascontiguousarray(w_rec_c),
            "bias": np.ascontiguousarray(bias_c),
        })
    return in_maps


def run(inputs, trace=False):
    if "nc" not in _cache:
        _cache["nc"] = _build()
    nc = _cache["nc"]
    in_maps = _prep_inputs(**inputs)
    res = bass_utils.run_bass_kernel_spmd(
        nc, in_maps, core_ids=list(range(NCORES)), trace=trace,
    )
    outputs = np.empty((B, S, H), np.float32)
    cells = np.empty((B, S, H), np.float32)
    for c in range(NCORES):
        U = slice(UC * c, UC * (c + 1))
        h = res.results[c]["h_out"]   # [NBLK, 128, 16, 32] = (blk, u, t, b)
        cc = res.results[c]["c_out"]
        outputs[:, :, U] = h.transpose(3, 0, 2, 1).reshape(B, S, 128)
        cells[:, :, U] = cc.transpose(3, 0, 2, 1).reshape(B, S, 128)
    return (outputs, cells), res


def kernel(**inputs):
    (outputs, cells), _ = run(inputs, trace=False)
    return outputs, cells



# revision 20
# speedup vs baseline: 1.1304x; 1.0512x over previous
"""Trainium2 Bass kernel for nn_CustomLSTM: B=32, S=512, D=512, H=1024.

Strategy (8 NeuronCores, one chip), v2:
  - Tensor-parallel over hidden units: core c owns H-units [128c, 128c+128)
    and computes all four gates for those units (gate order g, i, f, o).
  - No separate input-projection phase: each step's x-projection (4 gates x
    4 k-chunks, N=32) is computed by the PE into the step's PSUM bank during
    the previous step's broadcast dead time; x is streamed from DRAM in
    512-row blocks (16 steps per block). Biases fold into the ACT
    activations (per-partition bias operand).
  - Per step: 32 recurrent bf16 matmuls accumulate W_h^T @ h_{t-1} into one
    PSUM bank ([128 units, 4 gates x 32 batch]), gate-major so ACT overlaps
    the PE; ACT applies tanh/sigmoid per gate; DVE forms c and h; h (bf16)
    is exchanged between the 8 cores with remote_dma_broadcast.
  - Outputs (h, c) accumulate in 16-step SBUF rings and are written to DRAM
    once per 16 steps so the broadcast flight window stays clear of DMA
    descriptor traffic.
  - Raw bass (no Tile): hand-scheduled engine programs with explicit
    semaphores.

Self-contained: hardcodes all shapes; host side shards/reassembles.
"""
import numpy as np
import ml_dtypes

import concourse.bacc as bacc
import concourse.mybir as mybir
from concourse import bass_utils

F32 = mybir.dt.float32
BF16 = mybir.dt.bfloat16

B, S, D, H = 32, 512, 512, 1024
NCORES = 8
UC = H // NCORES          # units per core = 128

import os
if os.environ.get("KERNEL_SIM_STEPS"):
    S = int(os.environ["KERNEL_SIM_STEPS"])
NB = (S * B + 511) // 512   # x row blocks of 512 rows (16 steps each)
NBLK = (S + 15) // 16       # output blocks of 16 steps
ROWS = S * B

_cache = {}

# gate order everywhere: g(0), i(1), f(2), o(3)


def _build(detect_races=True):
    nc = bacc.Bacc(None, target_bir_lowering=False, num_devices=NCORES,
                   detect_race_conditions=detect_races)

    # ---------------- DRAM I/O ----------------
    xT_d = nc.dram_tensor("xT", [128, 4, ROWS], BF16, kind="ExternalInput")
    w_in_d = nc.dram_tensor("w_in", [128, 4, 512], BF16, kind="ExternalInput")
    w_rec_d = nc.dram_tensor("w_rec", [128, 8, 512], BF16, kind="ExternalInput")
    bias_d = nc.dram_tensor("bias", [128, 4], F32, kind="ExternalInput")
    h_out_d = nc.dram_tensor("h_out", [NBLK, 128, 16, 32], F32, kind="ExternalOutput")
    c_out_d = nc.dram_tensor("c_out", [NBLK, 128, 16, 32], F32, kind="ExternalOutput")

    # ---------------- SBUF ----------------
    xt_buf = nc.alloc_sbuf_tensor("xt_buf", [128, 3, 4, 512], BF16)
    w_in_sb = nc.alloc_sbuf_tensor("w_in_sb", [128, 4, 512], BF16)
    w_rec_sb = nc.alloc_sbuf_tensor("w_rec_sb", [128, 8, 512], BF16)
    bias_sb = nc.alloc_sbuf_tensor("bias_sb", [128, 4], F32)
    gact = nc.alloc_sbuf_tensor("gact", [128, 2, 4, 32], BF16)
    f_act = nc.alloc_sbuf_tensor("f_act", [128, 2, 32], F32)
    tmpu = nc.alloc_sbuf_tensor("tmpu", [128, 32], F32)
    tmpv = nc.alloc_sbuf_tensor("tmpv", [128, 32], F32)
    tc_sb = nc.alloc_sbuf_tensor("tc_sb", [128, 2, 32], BF16)
    c_ring = nc.alloc_sbuf_tensor("c_ring", [128, 16, 32], F32)
    h_ring = nc.alloc_sbuf_tensor("h_ring", [128, 16, 32], F32)
    hT_buf = nc.alloc_sbuf_tensor("hT_buf", [128, 2, 8, 32], BF16)
    h_send = nc.alloc_sbuf_tensor("h_send", [128, 2, 32], BF16)

    # bank p*4+g holds gate g of step parity p (cols 0:32)
    pp = nc.alloc_psum_tensor("pp", [128, 8, 512], F32)

    # ---------------- semaphores ----------------
    arr = [nc.alloc_semaphore(f"arr{j}") for j in range(NCORES)]
    loc_sem = nc.alloc_semaphore("loc")
    prep_sem = nc.alloc_semaphore("prep")
    dma_w = nc.alloc_semaphore("dma_w")
    xt_sem = nc.alloc_semaphore("xt_sem")
    xtf = nc.alloc_semaphore("xtf")          # x block consumed by PE
    pg = nc.alloc_semaphore("pg")            # psum gate done: 4 per step
    acts = nc.alloc_semaphore("acts")        # ACT gate ops done: 4 per step
    tcs = nc.alloc_semaphore("tcs")          # tanh(c) done: 1 per step
    cr = nc.alloc_semaphore("cr")            # c written to ring: 1 per step
    hr = nc.alloc_semaphore("hr")            # h staged to ring: 1 per step
    h_ready = nc.alloc_semaphore("h_ready")  # h_send written: 1 per step
    outc_sem = nc.alloc_semaphore("outc_sem")
    outh_sem = nc.alloc_semaphore("outh_sem")

    w_in = w_in_sb.ap()
    w_rec = w_rec_sb.ap()
    ppa = pp.ap()
    hT = hT_buf.ap()
    ga = gact.ap()

    def xpre_mms(pe, t):
        """x-projection matmuls for step t into bank t%2 (start of group)."""
        blk, r = t // 16, t % 16
        if r == 0:
            pe.wait_ge(xt_sem, 64 * (blk + 1))
        p = t % 2
        last = None
        for g in (0, 2, 1, 3):
            for k in range(4):
                last = nc.tensor.matmul(
                    ppa[:, p * 4 + g, 0:32],
                    w_in[:, k, g * 128:(g + 1) * 128],
                    xt_buf.ap()[:, blk % 3, k, r * 32:(r + 1) * 32],
                    start=(k == 0),
                    stop=(t == 0 and k == 3),
                )
            if t == 0:
                last.then_inc(pg, 1)
        if r == 15:
            last.then_inc(xtf, 1)

    with nc.Block() as block:

        # ================= SP: DMA feeder + batched output writer =========
        @block.sync
        def _(sp):
            sp.dma_start(w_in_sb.ap(), w_in_d.ap()).then_inc(dma_w, 16)
            sp.dma_start(w_rec_sb.ap(), w_rec_d.ap()).then_inc(dma_w, 16)
            sp.dma_start(bias_sb.ap(), bias_d.ap()).then_inc(dma_w, 16)
            for n in range(min(3, NB)):
                for q in range(4):
                    sp.dma_start(
                        xt_buf.ap()[:, n, q, :],
                        xT_d.ap()[:, q, n * 512:(n + 1) * 512],
                    ).then_inc(xt_sem, 16)
            for b in range(NBLK):
                sp.wait_ge(cr, 16 * b + 16)
                sp.dma_start(c_out_d.ap()[b], c_ring.ap()).then_inc(outc_sem, 16)
                sp.wait_ge(hr, 16 * b + 16)
                sp.dma_start(h_out_d.ap()[b], h_ring.ap()).then_inc(outh_sem, 16)
                # xt block b+3 reuses block b's slot (consumed at t=16b+14).
                # The 512KB load occupies the DMA engines ~1.4us, so split it
                # into 4 quarter-loads spread over 4 tail windows (t≡5,7,9,11)
                # to keep it out of the broadcast flights.
                n = b + 3
                if n < NB:
                    sp.wait_ge(xtf, b + 1)
                    for q in range(4):
                        sp.wait_ge(cr, 16 * b + 22 + 2 * q)
                        sp.dma_start(
                            xt_buf.ap()[:, n % 3, q, :],
                            xT_d.ap()[:, q, n * 512:(n + 1) * 512],
                        ).then_inc(xt_sem, 16)

        # ================= PE =================
        @block.tensor
        def _(pe):
            pe.wait_ge(dma_w, 48)
            # settle delay: the first broadcast otherwise fires ~5us after
            # kernel start and races device/engine startup, corrupting the
            # first steps (the old phase-1 gave the baseline ~250us of grace)
            if not os.environ.get("KERNEL_SIM_STEPS"):
                for _ in range(2):
                    pe.nop(cycle_cnt=60000)
            # prologue: x-projections for steps 0 and 1
            xpre_mms(pe, 0)
            if S > 1:
                xpre_mms(pe, 1)
            for t in range(1, S):
                p = t % 2
                # recurrent matmuls, issue order g, f, i, o: f before i
                # so the DVE's v=f*c_prev starts one ACT op earlier
                for g in (0, 2, 1, 3):
                    for k in range(NCORES):
                        if g == 0:
                            pe.wait_ge(arr[k], 2 * t)
                        ins = nc.tensor.matmul(
                            ppa[:, p * 4 + g, 0:32],
                            w_rec[:, k, g * 128:(g + 1) * 128],
                            hT[:, (t - 1) % 2, k, :],
                            start=False,
                            stop=(k == NCORES - 1),
                        )
                    ins.then_inc(pg, 1)
                # x-projection for step t+1 (runs in step t's dead time);
                # bank (t+1)%2 must have been drained by ACT at step t-1
                if t + 1 < S:
                    pe.wait_ge(acts, 4 * (t - 1) + 4)
                    xpre_mms(pe, t + 1)

        # ================= ACT =================
        @block.scalar
        def _(act):
            funcs = {
                0: mybir.ActivationFunctionType.Tanh,     # g
                1: mybir.ActivationFunctionType.Sigmoid,  # i
                2: mybir.ActivationFunctionType.Sigmoid,  # f
                3: mybir.ActivationFunctionType.Sigmoid,  # o
            }
            for t in range(S):
                p = t % 2
                for pos, g in enumerate((0, 2, 1, 3)):
                    act.wait_ge(pg, 4 * t + pos + 1)
                    # f (g==2) stays f32: it multiplies the f32 c_prev on
                    # DVE and mixed-dtype DVE inputs are risky on HW
                    out = f_act.ap()[:, p, :] if g == 2 else ga[:, p, g, :]
                    nc.scalar.activation(
                        out,
                        ppa[:, p * 4 + g, 0:32],
                        funcs[g],
                        bias=bias_sb.ap()[:, g:g + 1],
                    ).then_inc(acts, 1)
                act.wait_ge(cr, t + 1)
                nc.scalar.activation(
                    tc_sb.ap()[:, t % 2, :],
                    c_ring.ap()[:, t % 16, :],
                    mybir.ActivationFunctionType.Tanh,
                ).then_inc(tcs, 1)

        # ================= DVE =================
        @block.vector
        def _(dve):
            dve.memset(c_ring.ap()[:, 15, :], 0.0)
            dve.drain()
            for t in range(S):
                p = t % 2
                if t % 16 == 0 and t >= 16:
                    dve.wait_ge(outc_sem, 16 * (t // 16))
                # stage h_{t-1} = o_{t-1} * tanh(c_{t-1}) to the ring
                if t >= 1:
                    if (t - 1) % 16 == 0 and t - 1 >= 16:
                        dve.wait_ge(outh_sem, 16 * ((t - 1) // 16))
                    dve.wait_ge(tcs, t)
                    nc.vector.tensor_mul(
                        h_ring.ap()[:, (t - 1) % 16, :],
                        ga[:, (t - 1) % 2, 3, :],
                        tc_sb.ap()[:, (t - 1) % 2, :],
                    ).then_inc(hr, 1)
                # v = f * c_prev   (acts: g~ is 1st, sigma-f is 2nd)
                dve.wait_ge(acts, 4 * t + 2)
                nc.vector.tensor_mul(
                    tmpv.ap(), f_act.ap()[:, p, :], c_ring.ap()[:, (t - 1) % 16, :]
                )
                # u = i * g~   (sigma-i is 3rd)
                dve.wait_ge(acts, 4 * t + 3)
                nc.vector.tensor_mul(tmpu.ap(), ga[:, p, 1, :], ga[:, p, 0, :])
                dve.drain()
                # c_t = u + v
                nc.vector.tensor_add(
                    c_ring.ap()[:, t % 16, :], tmpu.ap(), tmpv.ap()
                ).then_inc(cr, 1)
                # h_send = o * tanh(c) (bf16) for the broadcast
                if t < S - 1:
                    if t >= 2:
                        dve.wait_ge(loc_sem, 16 * (t - 1))
                    dve.wait_ge(acts, 4 * t + 4)
                    dve.wait_ge(tcs, t + 1)
                    nc.vector.tensor_mul(
                        h_send.ap()[:, p, :], ga[:, p, 3, :], tc_sb.ap()[:, p, :]
                    ).then_inc(h_ready, 1)
            # epilogue: stage h_{S-1}
            dve.wait_ge(tcs, S)
            nc.vector.tensor_mul(
                h_ring.ap()[:, (S - 1) % 16, :],
                ga[:, (S - 1) % 2, 3, :],
                tc_sb.ap()[:, (S - 1) % 2, :],
            ).then_inc(hr, 1)

        # ================= Pool: remote all-gather =================
        @block.gpsimd
        def _(g):
            g.bir_kernel_barrier_wait([list(range(NCORES))])
            pid_reg = g.to_reg(g.partition_id())
            for kcore in range(NCORES):
                with g.If_eq(pid_reg, kcore):
                    for t in range(S - 1):
                        if t >= 1:
                            # descriptor-carveout reclaim: broadcast t-1's
                            # descs must be consumed before regenerating
                            g.wait_ge(loc_sem, 16 * t)
                        g.remote_dma_broadcast(
                            out_ap=hT[:, t % 2, kcore, :],
                            in_ap=h_send.ap()[:, t % 2, :],
                            remote_sem=arr[kcore],
                            local_sem=loc_sem,
                            rdests=[(0, j) for j in range(NCORES)],
                        ).then_inc(prep_sem, 1)
                        g.wait_ge(prep_sem, t + 1)
                        # fire on c_t completion: the trigger-exec (~0.33us)
                        # plus SWDGE doorbell (~0.68us) covers the remaining
                        # tanh(c) ACT + h=o*tanh(c) DVE chain (~0.69us), so
                        # the send fully overlaps the tail. Margin ~0.3us
                        # measured on HW traces (h_send written @cr+0.69,
                        # first SDMA read of h_send @cr+1.0).
                        g.wait_ge(cr, t + 1)
                        g.trigger_dma(1)

    nc.finalize()
    return nc


def _prep_inputs(x, W_ii, W_if, W_ig, W_io, W_hi, W_hf, W_hg, W_ho,
                 b_i, b_f, b_g, b_o):
    bf = ml_dtypes.bfloat16
    # xT: [D, S, B] -> [4, 128, S*B] -> [128, 4, S*B]; rows ordered (s, b)
    xT = np.ascontiguousarray(
        x.transpose(2, 1, 0).reshape(4, 128, ROWS).transpose(1, 0, 2)
    ).astype(bf)

    in_maps = []
    for c in range(NCORES):
        U = slice(UC * c, UC * (c + 1))
        # gate order [g, i, f, o]
        w_in_c = np.concatenate(
            [W_ig[:, U], W_ii[:, U], W_if[:, U], W_io[:, U]], axis=1
        )  # [512, 512]
        w_in_c = w_in_c.reshape(4, 128, 512).transpose(1, 0, 2).astype(bf)
        w_rec_c = np.concatenate(
            [W_hg[:, U], W_hi[:, U], W_hf[:, U], W_ho[:, U]], axis=1
        )  # [1024, 512]
        w_rec_c = w_rec_c.reshape(8, 128, 512).transpose(1, 0, 2).astype(bf)
        bias_c = np.stack(
            [b_g[U], b_i[U], b_f[U], b_o[U]], axis=1
        ).astype(np.float32)  # [128, 4]
        in_maps.append({
            "xT": xT,
            "w_in": np.ascontiguousarray(w_in_c),
            "w_rec": np.ascontiguousarray(w_rec_c),
            "bias": np.ascontiguousarray(bias_c),
        })
    return in_maps


def run(inputs, trace=False):
    if "nc" not in _cache:
        _cache["nc"] = _build()
    nc = _cache["nc"]
    in_maps = _prep_inputs(**inputs)
    res = bass_utils.run_bass_kernel_spmd(
        nc, in_maps, core_ids=list(range(NCORES)), trace=trace,
    )
    outputs = np.empty((B, S, H), np.float32)
    cells = np.empty((B, S, H), np.float32)
    for c in range(NCORES):
        U = slice(UC * c, UC * (c + 1))
        h = res.results[c]["h_out"]   # [NBLK, 128, 16, 32] = (blk, u, t, b)
        cc = res.results[c]["c_out"]
        outputs[:, :, U] = h.transpose(3, 0, 2, 1).reshape(B, S, 128)
        cells[:, :, U] = cc.transpose(3, 0, 2, 1).reshape(B, S, 128)
    return (outputs, cells), res


def kernel(**inputs):
    (outputs, cells), _ = run(inputs, trace=False)
    return outputs, cells

